# revision 6
# baseline (speedup 1.0000x reference)
"""ChannelMerger kernel for 8x Trainium2 NeuronCores (Bass/Tile).

Computes, for eeg [B,T,C], positions [B,C,2], heads [O,D]:
    emb     = fourier_emb(positions)              # [B,C,D], D = 2*12*12
    scores  = einsum('bcd,od->boc', emb, heads)   # [B,O,C]
    weights = softmax(scores, axis=2)
    out     = einsum('bct,boc->bot', eeg_ct, weights).transpose -> [B,T,O]

Sharding: data-parallel over batch B=32 -> 4 batches per core on 8 cores.
All compute (fourier, matmuls, softmax, weighted sum) runs on-device; the
host only shards/reshapes inputs and pads constants.

Pipeline design (v3): the kernel is HBM-bound (16.8 MB eeg read + 4.2 MB
fp16 out write per core), so everything is organized to keep the DMA rings
saturated from ~8us (after the fixed NEFF preamble) to the end:
  - the two tiny operands the weights path needs first (fourier matmul
    rows + per-core positions) ride the SAME SWDGE ring as the eeg flood
    in ONE merged DMA queued FIRST; heads/identity ride HWDGE;
  - eeg loads are 2 MB chunks so the Q7 descriptor-issue rate ramps the
    flood quickly and all chunks fit the 8 DMA sem lanes with no gating;
  - the fourier phases come from small PE matmuls (hi/lo bf16 rows
    accumulated in fp32 PSUM), processed as two wide (1024-col) pairs;
  - range reduction is 2 fused DVE ops (magic-constant RNE + fused
    subtract); the resulting sign flip is folded into negated heads;
  - softmax uses exp(s) = (1+tanh(s/2))/(1-tanh(s/2)): tanh lives in the
    SAME activation-table set as sin, avoiding a 2.7us exp table switch;
  - outputs are stored as fp16 (host upcasts), halving store traffic, and
    are flushed per 2-group chunk so stores overlap the remaining loads.
"""

import numpy as np

import concourse.bacc as bacc
import concourse.mybir as mybir
import concourse.tile as tile

# ---------------------------------------------------------------- constants
B, T, C = 32, 8192, 128
O = 64
N_FREQS = 12
N_IJ = N_FREQS * N_FREQS          # 144
D = 2 * N_IJ                      # 288
MARGIN = 0.2
N_CORES = 8
BPC = B // N_CORES                # batches per core = 4
TGROUP = 1024                     # t rows per group
NGROUP = T // TGROUP              # 8
JI = 8                            # row interleave within a group
SCH = 2                           # groups per store chunk (0.5 MiB write)
NPRO = 3                          # groups transposed in the prologue
F32 = mybir.dt.float32
MAGIC = float(1.5 * 2.0 ** 23)    # fp32 add => round-to-nearest integer

# load chunks per batch: (start_group, n_groups). 4-group (2 MiB) chunks
# ramp the flood fast and need no sem-lane gating (9 chunks, 8 lanes);
# the tail is split finer so the last chunk's latency is small.
_CHUNKS = {b: [(0, 4), (4, 4)] for b in range(BPC - 1)}
_CHUNKS[BPC - 1] = [(0, 4), (4, 2), (6, 2)]


# ------------------------------------------------------------ host constants
def _host_constants(heads: np.ndarray):
    """Pure layout/padding transforms of `heads` + static tables."""
    import ml_dtypes
    BF16 = ml_dtypes.bfloat16
    width = 1.0 + 2.0 * MARGIN
    # Frequencies in TURNS (cycles): loc_rad = 2*pi * (pos_x*p_i + pos_y*p_j).
    # Working in turns lets the device reduce the phase with one magic-const
    # round + one fused subtract before the Sin table lookup.
    p = np.arange(N_FREQS, dtype=np.float64) / width

    # loc matmul weights: loc[ij, (b,c)] = x*p_i + y*p_j (+0.25 for cos),
    # an 8-partition matmul with hi/lo bf16 splits of p and pos.
    # lhsT rows pair with pos rows [xh, yh, xl, yl, xh, yh, 1, 0]:
    #   rows 0/1: ph_i/ph_j, rows 2/3: ph_i/ph_j (x lo/y lo),
    #   rows 4/5: pl_i/pl_j, row 6: 0 (sin) / 0.25 (cos), row 7: pad.
    # The dropped pl*xl term is <4e-5 turns.
    lps = np.zeros((8, 2 * 2 * 128), dtype=BF16)  # cols: [sin c0,c1 | cos c0,c1]
    for c in range(2):
        for k in range(128):
            ij = 128 * c + k
            if ij >= N_IJ:
                continue
            for half, pv in ((0, p[ij // N_FREQS]), (1, p[ij % N_FREQS])):
                ph = BF16(pv)
                pl = BF16(pv - np.float64(ph))
                for base in (c * 128, 256 + c * 128):  # sin block, cos block
                    lps[0 + half, base + k] = ph
                    lps[2 + half, base + k] = ph
                    lps[4 + half, base + k] = pl
    lps[6, 256:] = BF16(0.25)  # cos = sin(t + 1/4 turn)

    # headsT chunks [K=128, O] for the 4 embq chunks (cos0, cos1, sin0, sin1).
    # NEGATED: the device computes embq = -sin(true phase) (sign falls out of
    # the fused range reduction), so -heads restores the true scores.
    hn = (-heads).astype(np.float16)
    hti = np.zeros((128, 4 * O + 128), dtype=np.float16)
    hti[:, 0 * O:1 * O] = hn[:, 0:128].T               # cos ij 0..127
    hti[:16, 1 * O:2 * O] = hn[:, 128:144].T           # cos ij 128..143
    hti[:, 2 * O:3 * O] = hn[:, 144:272].T             # sin ij 0..127
    hti[:16, 3 * O:4 * O] = hn[:, 272:288].T           # sin ij 128..143
    hti[:, 4 * O:] = np.eye(128, dtype=np.float16)     # transpose identity
    return lps, hti


def _wcon(lps: np.ndarray, positions_core: np.ndarray) -> np.ndarray:
    """lps + [BPC,C,2] positions -> [8, 1024] bf16: [lps | xh yh xl yl xh yh 1 0]."""
    import ml_dtypes
    BF16 = ml_dtypes.bfloat16
    pos = positions_core.astype(np.float64) + MARGIN  # [BPC, C, 2]
    out = np.zeros((8, 1024), dtype=BF16)
    out[:, 0:512] = lps
    p6 = out[:, 512:].reshape(8, BPC, C)
    for s in range(2):  # x, y
        v = pos[:, :, s]
        vh = v.astype(BF16)
        vl = (v - vh.astype(np.float64)).astype(BF16)
        p6[0 + s] = vh
        p6[2 + s] = vl
        p6[4 + s] = vh
    p6[6] = BF16(1.0)
    return out


# ------------------------------------------------------------- device kernel
def _build_nc(debug=False):
    # Bacc (not plain Bass): finalize() runs generate_event_semaphores,
    # which splits multi-sem waits (TRN2 allows 1 wait per instruction).
    nc = bacc.Bacc()
    BF16 = mybir.dt.bfloat16
    F16 = mybir.dt.float16
    eeg = nc.declare_dram_parameter("eeg", [BPC, T, C], F32, isOutput=False)
    wcon = nc.declare_dram_parameter("wcon", [8, 1024], BF16, isOutput=False)
    hti = nc.declare_dram_parameter("hti", [128, 4 * O + 128], F16, isOutput=False)
    out = nc.declare_dram_parameter("out", [BPC, T, O], F16, isOutput=True)
    if debug:
        wt_out = nc.declare_dram_parameter("wt_out", [128, BPC * O], F32, isOutput=True)
        emb_out = nc.declare_dram_parameter("emb_out", [128, 4 * 128], F32, isOutput=True)

    TWO_PI = float(2.0 * np.pi)

    with tile.TileContext(nc) as tc:
        with tc.tile_pool(name="consts", bufs=1) as cpool:
            # The weights path's operands first, ONE DMA on the SWDGE ring
            # (FIFO ahead of the eeg flood); heads/identity on HWDGE.
            wcon_sb = cpool.tile([8, 1024], BF16)
            nc.gpsimd.dma_start(out=wcon_sb, in_=wcon[:, :])
            hti_sb = cpool.tile([128, 4 * O + 128], F16)
            nc.sync.dma_start(out=hti_sb, in_=hti[:, :])
            ht16_sb = hti_sb[:, 0:4 * O]
            ident16_sb = hti_sb[:, 4 * O:]

            # PE warm-up: nudge the HAM clock gate while the first DMAs land.
            wu_a = cpool.tile([128, 128], BF16)
            wu_b = cpool.tile([128, 256], BF16)
            nc.vector.memset(wu_a, 1.0)
            nc.vector.memset(wu_b, 1.0)
            with tc.tile_pool(name="wups", bufs=1, space="PSUM") as wups:
                wu_ps = wups.tile([128, 256], F32)
                for _ in range(4):
                    nc.tensor.matmul(out=wu_ps, lhsT=wu_a, rhs=wu_b,
                                     start=True, stop=True)

            # softmaxed channel weights, transposed: [C, O] per batch
            wt16_all = cpool.tile([128, BPC * O], F16)

            with (
                tc.tile_pool(name="ein", bufs=16) as ein,
                tc.tile_pool(name="wsb", bufs=1) as wsb,
                tc.tile_pool(name="ets", bufs=6) as ets,
                tc.tile_pool(name="osb", bufs=6) as osb,
                tc.tile_pool(name="wps", bufs=1, space="PSUM") as wps,
                tc.tile_pool(name="etp", bufs=3, space="PSUM") as etp,
            ):

                # Kick off all eeg loads, casting fp32->fp16 inline on the
                # SWDGE path (fp16 keeps 10 mantissa bits; quantization is
                # ~1e-3 of output scale). Loads ride SWDGE, stores HWDGE.
                e_tiles = {}
                for b in range(BPC):
                    for g0, ng in _CHUNKS[b]:
                        e_sb = ein.tile([128, ng * TGROUP], F16, tag="e",
                                        name=f"e_{b}_{g0}")
                        rows = eeg[b][g0 * TGROUP:(g0 + ng) * TGROUP]
                        eeg_r = rows.rearrange("(g p j) c -> p g (j c)", p=128, j=JI)
                        nc.gpsimd.dma_start(
                            out=e_sb.rearrange("p (g x) -> p g x", g=ng), in_=eeg_r
                        )
                        for g in range(g0, g0 + ng):
                            e_tiles[g * BPC + b] = (e_sb, g - g0)
                def e_group(b, g):
                    e_sb, off = e_tiles[g * BPC + b]
                    return e_sb[:, off * TGROUP:(off + 1) * TGROUP]

                # ---------- phase 0: fourier emb + scores + softmax --------
                # loc[k, b, c] = x*p_i + y*p_j (+0.25 for cos), in TURNS,
                # via 8-partition bf16 matmuls; processed as 2 wide pairs.
                # q mapping matches hti: 0=cos c0, 1=cos c1, 2=sin c0, 3=sin c1.
                pos_r = wcon_sb[:, 512:].rearrange("p (b c) -> p b c", b=BPC)
                embq = wsb.tile([128, 4, BPC, 128], F16, tag="embq")
                with tc.tile_pool(name="locp", bufs=1, space="PSUM") as locp:
                    for pair in range(2):  # 0: cos q0/q1, 1: sin q2/q3
                        lc = locp.tile([128, 2, BPC, C], F32, tag="lc")
                        for cc in range(2):
                            off = (256 if pair == 0 else 0) + cc * 128
                            nc.tensor.matmul(
                                out=lc[:, cc], lhsT=wcon_sb[:, off:off + 128],
                                rhs=pos_r, start=True, stop=True,
                            )
                        # a = t + MAGIC rounds t to nearest int (RNE);
                        # (a - MAGIC) - t = -r, r in [-0.5, 0.5].
                        # sin(2pi*t) = -Sin(2pi*(-r)); the -1 lives in hti.
                        aq = wsb.tile([128, 2, BPC, C], F32, tag="aq")
                        nc.vector.tensor_scalar_add(out=aq, in0=lc,
                                                    scalar1=MAGIC)
                        negr = wsb.tile([128, 2, BPC, C], F32, tag="negr")
                        nc.vector.scalar_tensor_tensor(
                            out=negr, in0=aq, scalar=MAGIC, in1=lc,
                            op0=mybir.AluOpType.subtract,
                            op1=mybir.AluOpType.subtract,
                        )
                        nc.scalar.activation(
                            out=embq[:, 2 * pair:2 * pair + 2], in_=negr,
                            func=mybir.ActivationFunctionType.Sin,
                            scale=TWO_PI, bias=0.0,
                        )
                if debug:
                    nc.sync.dma_start(
                        out=emb_out[:, :],
                        in_=embq[:, :, 0, :],
                    )

                # Prologue: transpose the first NPRO groups of batch 0 while
                # the softmax dependency chain runs, so the PE isn't idle
                # and the main loop starts the moment weights are ready.
                pro_ets = {}
                for g in range(NPRO):
                    eg = e_group(0, g)
                    et_ps = etp.tile([128, JI * 128], F16, tag="etps",
                                     name=f"pro_etps_{g}")
                    for j in range(JI):
                        nc.tensor.transpose(
                            out=et_ps[:, j * 128:(j + 1) * 128],
                            in_=eg[:, j * 128:(j + 1) * 128],
                            identity=ident16_sb,
                        )
                    et_sb = ets.tile([128, JI * 128], F16, tag="etsb",
                                     name=f"pro_etsb_{g}")
                    nc.vector.tensor_copy(out=et_sb, in_=et_ps)
                    pro_ets[g] = et_sb

                scores_ps = wps.tile([O, BPC, 128], F32, tag="scores")
                for b in range(BPC):
                    for q in range(4):
                        nc.tensor.matmul(
                            out=scores_ps[:, b, :],
                            lhsT=ht16_sb[:, q * O:(q + 1) * O],
                            rhs=embq[:, q, b, :],
                            start=(q == 0), stop=(q == 3),
                        )
                # softmax via tanh (same ACT table set as sin -- no 2.7us
                # table switch): exp(s) = (1+tanh(s/2))/(1-tanh(s/2)).
                # scores are bounded (|s| < ~6) so this is fp32-accurate,
                # and softmax normalizes away the remaining scale.
                th = wsb.tile([O, BPC, 128], F32, tag="th")
                nc.scalar.activation(
                    out=th, in_=scores_ps,
                    func=mybir.ActivationFunctionType.Tanh,
                    scale=0.5, bias=0.0,
                )
                den = wsb.tile([O, BPC, 128], F32, tag="den")
                nc.vector.tensor_scalar(
                    out=den, in0=th, scalar1=-1.0, scalar2=1.0,
                    op0=mybir.AluOpType.mult, op1=mybir.AluOpType.add,
                )
                rcp2 = wsb.tile([O, BPC, 128], F32, tag="rcp2")
                nc.vector.reciprocal(out=rcp2, in_=den)
                probs = wsb.tile([O, BPC, 128], F32, tag="probs")
                ssum = wsb.tile([O, BPC], F32, tag="ssum")
                for b in range(BPC):
                    nc.vector.scalar_tensor_tensor(
                        out=probs[:, b, :], in0=th[:, b, :], scalar=1.0,
                        in1=rcp2[:, b, :],
                        op0=mybir.AluOpType.add, op1=mybir.AluOpType.mult,
                        accum_out=ssum[:, b:b + 1],
                    )

                rcp = wsb.tile([O, BPC], F32, tag="rcp")
                nc.vector.reciprocal(out=rcp, in_=ssum)
                wgt = wsb.tile([O, BPC, 128], F16, tag="wgt")
                wt_ps = wps.tile([128, BPC, O], F16, tag="wtps")
                for b in range(BPC):
                    nc.vector.tensor_scalar_mul(
                        out=wgt[:, b, :], in0=probs[:, b, :],
                        scalar1=rcp[:, b:b + 1],
                    )
                    nc.tensor.transpose(
                        out=wt_ps[:, b, :], in_=wgt[:, b, :],
                        identity=ident16_sb[0:O, 0:O],
                    )
                nc.vector.tensor_copy(out=wt16_all, in_=wt_ps)
                if debug:
                    nc.gpsimd.dma_start(out=wt_out[:, :], in_=wt16_all)

                # ---------- main loop: out[t,o] = sum_c eeg[t,c]*w[o,c] ----
                # Per 2-group store chunk: transpose eeg tiles on the PE,
                # matmul against the per-batch weights, cast to fp16 on the
                # ACT copy out of PSUM, store 0.5 MB on HWDGE.
                with tc.tile_pool(name="otp", bufs=3, space="PSUM") as otp:
                    for b in range(BPC):
                        wt_b = wt16_all[:, b * O:(b + 1) * O]
                        for ch in range(NGROUP // SCH):
                            o_sb = osb.tile([128, SCH * JI * O], F16, tag="osb",
                                            name=f"o_{b}_{ch}")
                            for g2 in range(SCH):
                                g = ch * SCH + g2
                                out_ps = otp.tile([128, JI * O], F32, tag="outps")
                                if b == 0 and g < NPRO:
                                    et_sb = pro_ets[g]  # prologue transpose
                                else:
                                    eg = e_group(b, g)
                                    et_ps = etp.tile([128, JI * 128], F16,
                                                     tag="etps")
                                    for j in range(JI):
                                        nc.tensor.transpose(
                                            out=et_ps[:, j * 128:(j + 1) * 128],
                                            in_=eg[:, j * 128:(j + 1) * 128],
                                            identity=ident16_sb,
                                        )
                                    et_sb = ets.tile([128, JI * 128], F16,
                                                     tag="etsb")
                                    nc.vector.tensor_copy(out=et_sb, in_=et_ps)
                                for j in range(JI):
                                    nc.tensor.matmul(
                                        out=out_ps[:, j * O:(j + 1) * O],
                                        lhsT=et_sb[:, j * 128:(j + 1) * 128],
                                        rhs=wt_b,
                                        start=True, stop=True,
                                    )
                                nc.scalar.copy(
                                    out=o_sb[:, g2 * JI * O:(g2 + 1) * JI * O],
                                    in_=out_ps,
                                )
                            out_r = out[b].rearrange(
                                "(g p j) o -> p g (j o)", p=128, j=JI
                            )
                            nc.sync.dma_start(
                                out=out_r[:, ch * SCH:(ch + 1) * SCH, :],
                                in_=o_sb.rearrange("p (g x) -> p g x", g=SCH),
                            )
    nc.finalize()
    return nc


_NC_CACHE = None


def _get_nc():
    global _NC_CACHE
    if _NC_CACHE is None:
        _NC_CACHE = _build_nc()
    return _NC_CACHE


def _make_in_maps(eeg, positions, heads):
    lps, hti = _host_constants(np.asarray(heads, dtype=np.float32))
    eeg = np.asarray(eeg, dtype=np.float32)
    positions = np.asarray(positions, dtype=np.float32)
    in_maps = []
    for core in range(N_CORES):
        sl = slice(core * BPC, (core + 1) * BPC)
        in_maps.append({
            "eeg": np.ascontiguousarray(eeg[sl]),
            "wcon": _wcon(lps, positions[sl]),
            "hti": hti,
        })
    return in_maps


def kernel(eeg, positions, heads, sub=None, **_unused):
    from concourse.bass_utils import run_bass_kernel_spmd

    nc = _get_nc()
    in_maps = _make_in_maps(eeg, positions, heads)
    res = run_bass_kernel_spmd(nc, in_maps, list(range(N_CORES)))
    out = np.concatenate([res.results[c]["out"] for c in range(N_CORES)], axis=0)
    return out.astype(np.float32)


# revision 10
# speedup vs baseline: 1.0299x; 1.0299x over previous
"""ChannelMerger kernel for 8x Trainium2 NeuronCores (Bass/Tile).

Computes, for eeg [B,T,C], positions [B,C,2], heads [O,D]:
    emb     = fourier_emb(positions)              # [B,C,D], D = 2*12*12
    scores  = einsum('bcd,od->boc', emb, heads)   # [B,O,C]
    weights = softmax(scores, axis=2)
    out     = einsum('bct,boc->bot', eeg_ct, weights).transpose -> [B,T,O]

Sharding: data-parallel over batch B=32 -> 4 batches per core on 8 cores.
All compute (fourier, matmuls, softmax, weighted sum) runs on-device; the
host only shards/reshapes inputs and pads constants.

Pipeline design (v3): the kernel is HBM-bound (16.8 MB eeg read + 4.2 MB
fp16 out write per core), so everything is organized to keep the DMA rings
saturated from ~8us (after the fixed NEFF preamble) to the end:
  - the two tiny operands the weights path needs first (fourier matmul
    rows + per-core positions) ride the SAME SWDGE ring as the eeg flood
    in ONE merged DMA queued FIRST; heads/identity ride HWDGE;
  - eeg loads are 2 MB chunks so the Q7 descriptor-issue rate ramps the
    flood quickly and all chunks fit the 8 DMA sem lanes with no gating;
  - the fourier phases come from small PE matmuls (hi/lo bf16 rows
    accumulated in fp32 PSUM), processed as two wide (1024-col) pairs;
  - range reduction is 2 fused DVE ops (magic-constant RNE + fused
    subtract); the resulting sign flip is folded into negated heads;
  - softmax uses exp(s) = (1+tanh(s/2))/(1-tanh(s/2)): tanh lives in the
    SAME activation-table set as sin, avoiding a 2.7us exp table switch;
  - outputs are stored as fp16 (host upcasts), halving store traffic, and
    are flushed per 2-group chunk so stores overlap the remaining loads.
"""

import numpy as np

import concourse.bacc as bacc
import concourse.mybir as mybir
import concourse.tile as tile

# ---------------------------------------------------------------- constants
B, T, C = 32, 8192, 128
O = 64
N_FREQS = 12
N_IJ = N_FREQS * N_FREQS          # 144
D = 2 * N_IJ                      # 288
MARGIN = 0.2
N_CORES = 8
BPC = B // N_CORES                # batches per core = 4
TGROUP = 1024                     # t rows per group
NGROUP = T // TGROUP              # 8
JI = 8                            # row interleave within a group
SCH = 2                           # groups per store chunk (0.5 MiB write)
NPRO = 2                          # groups transposed in the prologue
F32 = mybir.dt.float32
MAGIC = float(1.5 * 2.0 ** 23)    # fp32 add => round-to-nearest integer

# load chunks per batch: (start_group, n_groups). 4-group (2 MiB) chunks
# ramp the flood fast and need no sem-lane gating (9 chunks, 8 lanes);
# the tail is split finer so the last chunk's latency is small.
_CHUNKS = {b: [(0, 4), (4, 4)] for b in range(BPC - 1)}
_CHUNKS[BPC - 1] = [(0, 4), (4, 2), (6, 2)]


# ------------------------------------------------------------ host constants
def _host_constants(heads: np.ndarray):
    """Pure layout/padding transforms of `heads` + static tables."""
    import ml_dtypes
    BF16 = ml_dtypes.bfloat16
    width = 1.0 + 2.0 * MARGIN
    # Frequencies in TURNS (cycles): loc_rad = 2*pi * (pos_x*p_i + pos_y*p_j).
    # Working in turns lets the device reduce the phase with one magic-const
    # round + one fused subtract before the Sin table lookup.
    p = np.arange(N_FREQS, dtype=np.float64) / width

    # loc matmul weights: loc[ij, (b,c)] = x*p_i + y*p_j (+0.25 for cos),
    # an 8-partition matmul with hi/lo bf16 splits of p and pos.
    # lhsT rows pair with pos rows [xh, yh, xl, yl, xh, yh, 1, 0]:
    #   rows 0/1: ph_i/ph_j, rows 2/3: ph_i/ph_j (x lo/y lo),
    #   rows 4/5: pl_i/pl_j, row 6: 0 (sin) / 0.25 (cos), row 7: pad.
    # The dropped pl*xl term is <4e-5 turns.
    lps = np.zeros((8, 2 * 2 * 128), dtype=BF16)  # cols: [sin c0,c1 | cos c0,c1]
    for c in range(2):
        for k in range(128):
            ij = 128 * c + k
            if ij >= N_IJ:
                continue
            for half, pv in ((0, p[ij // N_FREQS]), (1, p[ij % N_FREQS])):
                ph = BF16(pv)
                pl = BF16(pv - np.float64(ph))
                for base in (c * 128, 256 + c * 128):  # sin block, cos block
                    lps[0 + half, base + k] = ph
                    lps[2 + half, base + k] = ph
                    lps[4 + half, base + k] = pl
    lps[6, 256:] = BF16(0.25)  # cos = sin(t + 1/4 turn)

    # headsT chunks [K=128, O] for the 4 embq chunks (cos0, cos1, sin0, sin1).
    # NEGATED: the device computes embq = -sin(true phase) (sign falls out of
    # the fused range reduction), so -heads restores the true scores.
    hn = (-heads).astype(np.float16)
    hti = np.zeros((128, 4 * O + 128), dtype=np.float16)
    hti[:, 0 * O:1 * O] = hn[:, 0:128].T               # cos ij 0..127
    hti[:16, 1 * O:2 * O] = hn[:, 128:144].T           # cos ij 128..143
    hti[:, 2 * O:3 * O] = hn[:, 144:272].T             # sin ij 0..127
    hti[:16, 3 * O:4 * O] = hn[:, 272:288].T           # sin ij 128..143
    hti[:, 4 * O:] = np.eye(128, dtype=np.float16)     # transpose identity
    return lps, hti


def _wcon(lps: np.ndarray, positions_core: np.ndarray) -> np.ndarray:
    """lps + [BPC,C,2] positions -> [8, 1024] bf16: [lps | xh yh xl yl xh yh 1 0]."""
    import ml_dtypes
    BF16 = ml_dtypes.bfloat16
    pos = positions_core.astype(np.float64) + MARGIN  # [BPC, C, 2]
    out = np.zeros((8, 1024), dtype=BF16)
    out[:, 0:512] = lps
    p6 = out[:, 512:].reshape(8, BPC, C)
    for s in range(2):  # x, y
        v = pos[:, :, s]
        vh = v.astype(BF16)
        vl = (v - vh.astype(np.float64)).astype(BF16)
        p6[0 + s] = vh
        p6[2 + s] = vl
        p6[4 + s] = vh
    p6[6] = BF16(1.0)
    return out


# ------------------------------------------------------------- device kernel
def _build_nc(debug=False):
    # Bacc (not plain Bass): finalize() runs generate_event_semaphores,
    # which splits multi-sem waits (TRN2 allows 1 wait per instruction).
    nc = bacc.Bacc()
    BF16 = mybir.dt.bfloat16
    F16 = mybir.dt.float16
    eeg = nc.declare_dram_parameter("eeg", [BPC, T, C], F32, isOutput=False)
    wcon = nc.declare_dram_parameter("wcon", [8, 1024], BF16, isOutput=False)
    hti = nc.declare_dram_parameter("hti", [128, 4 * O + 128], F16, isOutput=False)
    out = nc.declare_dram_parameter("out", [BPC, T, O], F16, isOutput=True)
    if debug:
        wt_out = nc.declare_dram_parameter("wt_out", [128, BPC * O], F32, isOutput=True)
        emb_out = nc.declare_dram_parameter("emb_out", [128, 4 * 128], F32, isOutput=True)

    TWO_PI = float(2.0 * np.pi)

    with tile.TileContext(nc) as tc:
        with tc.tile_pool(name="consts", bufs=1) as cpool:
            # The weights path's operands first, ONE DMA on the SWDGE ring
            # (FIFO ahead of the eeg flood); heads/identity on HWDGE.
            wcon_sb = cpool.tile([8, 1024], BF16)
            nc.gpsimd.dma_start(out=wcon_sb, in_=wcon[:, :])
            hti_sb = cpool.tile([128, 4 * O + 128], F16)
            nc.sync.dma_start(out=hti_sb, in_=hti[:, :])
            ht16_sb = hti_sb[:, 0:4 * O]
            ident16_sb = hti_sb[:, 4 * O:]

            # PE warm-up: nudge the HAM clock gate while the first DMAs land.
            wu_a = cpool.tile([128, 128], BF16)
            wu_b = cpool.tile([128, 256], BF16)
            nc.vector.memset(wu_a, 1.0)
            nc.vector.memset(wu_b, 1.0)
            with tc.tile_pool(name="wups", bufs=1, space="PSUM") as wups:
                wu_ps = wups.tile([128, 256], F32)
                for _ in range(4):
                    nc.tensor.matmul(out=wu_ps, lhsT=wu_a, rhs=wu_b,
                                     start=True, stop=True)

            # softmaxed channel weights, transposed: [C, O] per batch
            wt16_all = cpool.tile([128, BPC * O], F16)

            with (
                tc.tile_pool(name="ein", bufs=16) as ein,
                tc.tile_pool(name="wsb", bufs=1) as wsb,
                tc.tile_pool(name="ets", bufs=6) as ets,
                tc.tile_pool(name="osb", bufs=6) as osb,
                tc.tile_pool(name="wps", bufs=1, space="PSUM") as wps,
                tc.tile_pool(name="etp", bufs=2, space="PSUM") as etp,
            ):

                # Kick off all eeg loads, casting fp32->fp16 inline on the
                # SWDGE path (fp16 keeps 10 mantissa bits; quantization is
                # ~1e-3 of output scale). Loads ride SWDGE, stores HWDGE.
                e_tiles = {}
                for b in range(BPC):
                    for g0, ng in _CHUNKS[b]:
                        e_sb = ein.tile([128, ng * TGROUP], F16, tag="e",
                                        name=f"e_{b}_{g0}")
                        rows = eeg[b][g0 * TGROUP:(g0 + ng) * TGROUP]
                        eeg_r = rows.rearrange("(g p j) c -> p g (j c)", p=128, j=JI)
                        nc.gpsimd.dma_start(
                            out=e_sb.rearrange("p (g x) -> p g x", g=ng), in_=eeg_r
                        )
                        for g in range(g0, g0 + ng):
                            e_tiles[g * BPC + b] = (e_sb, g - g0)
                def e_group(b, g):
                    e_sb, off = e_tiles[g * BPC + b]
                    return e_sb[:, off * TGROUP:(off + 1) * TGROUP]

                # ---------- phase 0: fourier emb + scores + softmax --------
                # loc[k, b, c] = x*p_i + y*p_j (+0.25 for cos), in TURNS,
                # via 8-partition bf16 matmuls; processed as 2 wide pairs.
                # q mapping matches hti: 0=cos c0, 1=cos c1, 2=sin c0, 3=sin c1.
                pos_r = wcon_sb[:, 512:].rearrange("p (b c) -> p b c", b=BPC)
                embq = wsb.tile([128, 4, BPC, 128], F16, tag="embq")
                with tc.tile_pool(name="locp", bufs=2, space="PSUM") as locp:
                    for pair in range(2):  # 0: cos q0/q1, 1: sin q2/q3
                        lc = locp.tile([128, 2, BPC, C], F32, tag="lc")
                        for cc in range(2):
                            off = (256 if pair == 0 else 0) + cc * 128
                            nc.tensor.matmul(
                                out=lc[:, cc], lhsT=wcon_sb[:, off:off + 128],
                                rhs=pos_r, start=True, stop=True,
                            )
                        # a = t + MAGIC rounds t to nearest int (RNE);
                        # (a - MAGIC) - t = -r, r in [-0.5, 0.5].
                        # sin(2pi*t) = -Sin(2pi*(-r)); the -1 lives in hti.
                        aq = wsb.tile([128, 2, BPC, C], F32, tag="aq")
                        nc.vector.tensor_scalar_add(out=aq, in0=lc,
                                                    scalar1=MAGIC)
                        negr = wsb.tile([128, 2, BPC, C], F32, tag="negr")
                        nc.vector.scalar_tensor_tensor(
                            out=negr, in0=aq, scalar=MAGIC, in1=lc,
                            op0=mybir.AluOpType.subtract,
                            op1=mybir.AluOpType.subtract,
                        )
                        nc.scalar.activation(
                            out=embq[:, 2 * pair:2 * pair + 2], in_=negr,
                            func=mybir.ActivationFunctionType.Sin,
                            scale=TWO_PI, bias=0.0,
                        )
                if debug:
                    nc.sync.dma_start(
                        out=emb_out[:, :],
                        in_=embq[:, :, 0, :],
                    )

                # Prologue: transpose the first NPRO groups of batch 0 while
                # the softmax dependency chain runs, so the PE isn't idle
                # and the main loop starts the moment weights are ready.
                pro_ets = {}
                for g in range(NPRO):
                    eg = e_group(0, g)
                    et_ps = etp.tile([128, JI * 128], F16, tag="etps",
                                     name=f"pro_etps_{g}")
                    for j in range(JI):
                        nc.tensor.transpose(
                            out=et_ps[:, j * 128:(j + 1) * 128],
                            in_=eg[:, j * 128:(j + 1) * 128],
                            identity=ident16_sb,
                        )
                    et_sb = ets.tile([128, JI * 128], F16, tag="etsb",
                                     name=f"pro_etsb_{g}")
                    nc.vector.tensor_copy(out=et_sb, in_=et_ps)
                    pro_ets[g] = et_sb

                scores_ps = wps.tile([O, BPC, 128], F32, tag="scores")
                for b in range(BPC):
                    for q in range(4):
                        nc.tensor.matmul(
                            out=scores_ps[:, b, :],
                            lhsT=ht16_sb[:, q * O:(q + 1) * O],
                            rhs=embq[:, q, b, :],
                            start=(q == 0), stop=(q == 3),
                        )
                # scores are bounded (|s| < ~10): plain exp is fp32-safe and
                # softmax is shift-invariant, so skip the max-subtraction.
                # The exp table switch (~2.7us) overlaps the scores matmuls.
                probs = wsb.tile([O, BPC, 128], F32, tag="probs")
                ssum = wsb.tile([O, BPC], F32, tag="ssum")
                for b in range(BPC):
                    nc.scalar.activation(
                        out=probs[:, b, :], in_=scores_ps[:, b, :],
                        func=mybir.ActivationFunctionType.Exp,
                        bias=0.0, accum_out=ssum[:, b:b + 1],
                    )

                rcp = wsb.tile([O, BPC], F32, tag="rcp")
                nc.vector.reciprocal(out=rcp, in_=ssum)
                wgt = wsb.tile([O, BPC, 128], F16, tag="wgt")
                wt_ps = wps.tile([128, BPC, O], F16, tag="wtps")
                for b in range(BPC):
                    nc.vector.tensor_scalar_mul(
                        out=wgt[:, b, :], in0=probs[:, b, :],
                        scalar1=rcp[:, b:b + 1],
                    )
                    nc.tensor.transpose(
                        out=wt_ps[:, b, :], in_=wgt[:, b, :],
                        identity=ident16_sb[0:O, 0:O],
                    )
                nc.vector.tensor_copy(out=wt16_all, in_=wt_ps)
                if debug:
                    nc.gpsimd.dma_start(out=wt_out[:, :], in_=wt16_all)

                # ---------- main loop: out[t,o] = sum_c eeg[t,c]*w[o,c] ----
                # Per 2-group store chunk: transpose eeg tiles on the PE,
                # matmul against the per-batch weights, cast to fp16 on the
                # ACT copy out of PSUM, store 0.5 MB on HWDGE.
                with tc.tile_pool(name="otp", bufs=3, space="PSUM") as otp:
                    for b in range(BPC):
                        wt_b = wt16_all[:, b * O:(b + 1) * O]
                        for ch in range(NGROUP // SCH):
                            o_sb = osb.tile([128, SCH * JI * O], F16, tag="osb",
                                            name=f"o_{b}_{ch}")
                            for g2 in range(SCH):
                                g = ch * SCH + g2
                                out_ps = otp.tile([128, JI * O], F32, tag="outps")
                                if b == 0 and g < NPRO:
                                    et_sb = pro_ets[g]  # prologue transpose
                                else:
                                    eg = e_group(b, g)
                                    et_ps = etp.tile([128, JI * 128], F16,
                                                     tag="etps")
                                    for j in range(JI):
                                        nc.tensor.transpose(
                                            out=et_ps[:, j * 128:(j + 1) * 128],
                                            in_=eg[:, j * 128:(j + 1) * 128],
                                            identity=ident16_sb,
                                        )
                                    et_sb = ets.tile([128, JI * 128], F16,
                                                     tag="etsb")
                                    nc.vector.tensor_copy(out=et_sb, in_=et_ps)
                                for j in range(JI):
                                    nc.tensor.matmul(
                                        out=out_ps[:, j * O:(j + 1) * O],
                                        lhsT=et_sb[:, j * 128:(j + 1) * 128],
                                        rhs=wt_b,
                                        start=True, stop=True,
                                    )
                                nc.scalar.copy(
                                    out=o_sb[:, g2 * JI * O:(g2 + 1) * JI * O],
                                    in_=out_ps,
                                )
                            out_r = out[b].rearrange(
                                "(g p j) o -> p g (j o)", p=128, j=JI
                            )
                            nc.sync.dma_start(
                                out=out_r[:, ch * SCH:(ch + 1) * SCH, :],
                                in_=o_sb.rearrange("p (g x) -> p g x", g=SCH),
                            )
    nc.finalize()
    return nc


_NC_CACHE = None


def _get_nc():
    global _NC_CACHE
    if _NC_CACHE is None:
        _NC_CACHE = _build_nc()
    return _NC_CACHE


def _make_in_maps(eeg, positions, heads):
    lps, hti = _host_constants(np.asarray(heads, dtype=np.float32))
    eeg = np.asarray(eeg, dtype=np.float32)
    positions = np.asarray(positions, dtype=np.float32)
    in_maps = []
    for core in range(N_CORES):
        sl = slice(core * BPC, (core + 1) * BPC)
        in_maps.append({
            "eeg": np.ascontiguousarray(eeg[sl]),
            "wcon": _wcon(lps, positions[sl]),
            "hti": hti,
        })
    return in_maps


def kernel(eeg, positions, heads, sub=None, **_unused):
    from concourse.bass_utils import run_bass_kernel_spmd

    nc = _get_nc()
    in_maps = _make_in_maps(eeg, positions, heads)
    res = run_bass_kernel_spmd(nc, in_maps, list(range(N_CORES)))
    out = np.concatenate([res.results[c]["out"] for c in range(N_CORES)], axis=0)
    return out.astype(np.float32)


# revision 16
# speedup vs baseline: 1.1190x; 1.0865x over previous
"""ChannelMerger kernel for 8x Trainium2 NeuronCores (Bass/Tile).

Computes, for eeg [B,T,C], positions [B,C,2], heads [O,D]:
    emb     = fourier_emb(positions)              # [B,C,D], D = 2*12*12
    scores  = einsum('bcd,od->boc', emb, heads)   # [B,O,C]
    weights = softmax(scores, axis=2)
    out     = einsum('bct,boc->bot', eeg_ct, weights).transpose -> [B,T,O]

Sharding: data-parallel over batch B=32 -> 4 batches per core on 8 cores.
All compute (fourier, matmuls, softmax, weighted sum) runs on-device; the
host only shards/reshapes inputs and pads constants.

Pipeline design (v3): the kernel is HBM-bound (16.8 MB eeg read + 4.2 MB
fp16 out write per core), so everything is organized to keep the DMA rings
saturated from ~8us (after the fixed NEFF preamble) to the end:
  - the two tiny operands the weights path needs first (fourier matmul
    rows + per-core positions) ride the SAME SWDGE ring as the eeg flood
    in ONE merged DMA queued FIRST; heads/identity ride HWDGE;
  - eeg loads are 2 MB chunks so the Q7 descriptor-issue rate ramps the
    flood quickly and all chunks fit the 8 DMA sem lanes with no gating;
  - the fourier phases come from small PE matmuls (hi/lo bf16 rows
    accumulated in fp32 PSUM), processed as two wide (1024-col) pairs;
  - range reduction is 2 fused DVE ops (magic-constant RNE + fused
    subtract); the resulting sign flip is folded into negated heads;
  - softmax uses exp(s) = (1+tanh(s/2))/(1-tanh(s/2)): tanh lives in the
    SAME activation-table set as sin, avoiding a 2.7us exp table switch;
  - outputs are stored as fp16 (host upcasts), halving store traffic, and
    are flushed per 2-group chunk so stores overlap the remaining loads.
"""

import numpy as np

import concourse.bacc as bacc
import concourse.mybir as mybir
import concourse.tile as tile

# ---------------------------------------------------------------- constants
B, T, C = 32, 8192, 128
O = 64
N_FREQS = 12
N_IJ = N_FREQS * N_FREQS          # 144
D = 2 * N_IJ                      # 288
MARGIN = 0.2
N_CORES = 8
BPC = B // N_CORES                # batches per core = 4
TGROUP = 1024                     # t rows per group
NGROUP = T // TGROUP              # 8
JI = 8                            # row interleave within a group
SCH = 2                           # groups per store chunk (0.5 MiB write)
NPRO = 2                          # groups transposed in the prologue
F32 = mybir.dt.float32
MAGIC = float(1.5 * 2.0 ** 23)    # fp32 add => round-to-nearest integer

# load chunks per batch: (start_group, n_groups). 4-group (2 MiB) chunks
# ramp the flood fast and need no sem-lane gating (9 chunks, 8 lanes);
# the tail is split finer so the last chunk's latency is small.
_CHUNKS = {b: [(0, 4), (4, 4)] for b in range(BPC - 1)}
_CHUNKS[BPC - 1] = [(0, 4), (4, 2), (6, 2)]


# ------------------------------------------------------------ host constants
def _host_constants(heads: np.ndarray):
    """Pure layout/padding transforms of `heads` + static tables."""
    import ml_dtypes
    BF16 = ml_dtypes.bfloat16
    width = 1.0 + 2.0 * MARGIN
    # Frequencies in TURNS (cycles): loc_rad = 2*pi * (pos_x*p_i + pos_y*p_j).
    # Working in turns lets the device reduce the phase with one magic-const
    # round + one fused subtract before the Sin table lookup.
    p = np.arange(N_FREQS, dtype=np.float64) / width

    # loc matmul weights: loc[ij, (b,c)] = x*p_i + y*p_j (+0.25 for cos),
    # an 8-partition matmul with hi/lo bf16 splits of p and pos.
    # lhsT rows pair with pos rows [xh, yh, xl, yl, xh, yh, 1, 0]:
    #   rows 0/1: ph_i/ph_j, rows 2/3: ph_i/ph_j (x lo/y lo),
    #   rows 4/5: pl_i/pl_j, row 6: 0 (sin) / 0.25 (cos), row 7: pad.
    # The dropped pl*xl term is <4e-5 turns.
    lps = np.zeros((8, 2 * 2 * 128), dtype=BF16)  # cols: [sin c0,c1 | cos c0,c1]
    for c in range(2):
        for k in range(128):
            ij = 128 * c + k
            if ij >= N_IJ:
                continue
            for half, pv in ((0, p[ij // N_FREQS]), (1, p[ij % N_FREQS])):
                ph = BF16(pv)
                pl = BF16(pv - np.float64(ph))
                for base in (c * 128, 256 + c * 128):  # sin block, cos block
                    lps[0 + half, base + k] = ph
                    lps[2 + half, base + k] = ph
                    lps[4 + half, base + k] = pl
    lps[6, 256:] = BF16(0.25)  # cos = sin(t + 1/4 turn)

    # headsT chunks [K=128, O] for the 4 embq chunks (cos0, cos1, sin0, sin1).
    # NEGATED: the device computes embq = -sin(true phase) (sign falls out of
    # the fused range reduction), so -heads restores the true scores.
    hn = (-heads).astype(np.float16)
    hti = np.zeros((128, 4 * O + 128), dtype=np.float16)
    hti[:, 0 * O:1 * O] = hn[:, 0:128].T               # cos ij 0..127
    hti[:16, 1 * O:2 * O] = hn[:, 128:144].T           # cos ij 128..143
    hti[:, 2 * O:3 * O] = hn[:, 144:272].T             # sin ij 0..127
    hti[:16, 3 * O:4 * O] = hn[:, 272:288].T           # sin ij 128..143
    hti[:, 4 * O:] = np.eye(128, dtype=np.float16)     # transpose identity
    return lps, hti


def _wcon(lps: np.ndarray, positions_core: np.ndarray) -> np.ndarray:
    """lps + [BPC,C,2] positions -> [8, 1024] bf16: [lps | xh yh xl yl xh yh 1 0]."""
    import ml_dtypes
    BF16 = ml_dtypes.bfloat16
    pos = positions_core.astype(np.float64) + MARGIN  # [BPC, C, 2]
    out = np.zeros((8, 1024), dtype=BF16)
    out[:, 0:512] = lps
    p6 = out[:, 512:].reshape(8, BPC, C)
    for s in range(2):  # x, y
        v = pos[:, :, s]
        vh = v.astype(BF16)
        vl = (v - vh.astype(np.float64)).astype(BF16)
        p6[0 + s] = vh
        p6[2 + s] = vl
        p6[4 + s] = vh
    p6[6] = BF16(1.0)
    return out


# ------------------------------------------------------------- device kernel
def _build_nc(debug=False):
    # Bacc (not plain Bass): finalize() runs generate_event_semaphores,
    # which splits multi-sem waits (TRN2 allows 1 wait per instruction).
    nc = bacc.Bacc()
    BF16 = mybir.dt.bfloat16
    F16 = mybir.dt.float16
    eeg = nc.declare_dram_parameter("eeg", [BPC, T, C], F32, isOutput=False)
    wcon = nc.declare_dram_parameter("wcon", [8, 1024], BF16, isOutput=False)
    hti = nc.declare_dram_parameter("hti", [128, 4 * O + 128], F16, isOutput=False)
    out = nc.declare_dram_parameter("out", [BPC, T, O], F16, isOutput=True)
    if debug:
        wt_out = nc.declare_dram_parameter("wt_out", [128, BPC * O], F32, isOutput=True)
        emb_out = nc.declare_dram_parameter("emb_out", [128, 4 * 128], F32, isOutput=True)

    TWO_PI = float(2.0 * np.pi)

    with tile.TileContext(nc) as tc:
        with tc.tile_pool(name="consts", bufs=1) as cpool:
            # Constants ride the HWDGE ring, which the eeg flood (SWDGE)
            # never touches, so they land by ~8.5us; the gpsimd queue is
            # free to start the eeg flood immediately.
            wcon_sb = cpool.tile([8, 1024], BF16)
            nc.sync.dma_start(out=wcon_sb, in_=wcon[:, :])
            hti_sb = cpool.tile([128, 4 * O + 128], F16)
            nc.sync.dma_start(out=hti_sb, in_=hti[:, :])
            ht16_sb = hti_sb[:, 0:4 * O]
            ident16_sb = hti_sb[:, 4 * O:]

            # PE warm-up: nudge the HAM clock gate while the first DMAs land.
            wu_a = cpool.tile([128, 128], BF16)
            wu_b = cpool.tile([128, 256], BF16)
            nc.vector.memset(wu_a, 1.0)
            nc.vector.memset(wu_b, 1.0)
            magic_sb = cpool.tile([128, 1], F32)  # ACT bias for the RNE add
            nc.vector.memset(magic_sb, MAGIC)
            with tc.tile_pool(name="wups", bufs=1, space="PSUM") as wups:
                wu_ps = wups.tile([128, 256], F32)
                for _ in range(4):
                    nc.tensor.matmul(out=wu_ps, lhsT=wu_a, rhs=wu_b,
                                     start=True, stop=True)

            # softmaxed channel weights, transposed: [C, O] per batch
            wt16_all = cpool.tile([128, BPC * O], F16)

            with (
                tc.tile_pool(name="ein", bufs=16) as ein,
                tc.tile_pool(name="wsb", bufs=1) as wsb,
                tc.tile_pool(name="ets", bufs=6) as ets,
                tc.tile_pool(name="osb", bufs=6) as osb,
                tc.tile_pool(name="wps", bufs=1, space="PSUM") as wps,
                tc.tile_pool(name="etp", bufs=2, space="PSUM") as etp,
            ):

                # Kick off all eeg loads, casting fp32->fp16 inline on the
                # SWDGE path (fp16 keeps 10 mantissa bits; quantization is
                # ~1e-3 of output scale). Loads ride SWDGE, stores HWDGE.
                e_tiles = {}
                for b in range(BPC):
                    for g0, ng in _CHUNKS[b]:
                        e_sb = ein.tile([128, ng * TGROUP], F16, tag="e",
                                        name=f"e_{b}_{g0}")
                        rows = eeg[b][g0 * TGROUP:(g0 + ng) * TGROUP]
                        eeg_r = rows.rearrange("(g p j) c -> p g (j c)", p=128, j=JI)
                        nc.gpsimd.dma_start(
                            out=e_sb.rearrange("p (g x) -> p g x", g=ng), in_=eeg_r
                        )
                        for g in range(g0, g0 + ng):
                            e_tiles[g * BPC + b] = (e_sb, g - g0)
                def e_group(b, g):
                    e_sb, off = e_tiles[g * BPC + b]
                    return e_sb[:, off * TGROUP:(off + 1) * TGROUP]

                # ---------- phase 0: fourier emb + scores + softmax --------
                # loc[k, b, c] = x*p_i + y*p_j (+0.25 for cos), in TURNS,
                # via 8-partition bf16 matmuls; processed as 2 wide pairs.
                # q mapping matches hti: 0=cos c0, 1=cos c1, 2=sin c0, 3=sin c1.
                pos_r = wcon_sb[:, 512:].rearrange("p (b c) -> p b c", b=BPC)
                embq = wsb.tile([128, 4, BPC, 128], F16, tag="embq")
                with tc.tile_pool(name="locp", bufs=2, space="PSUM") as locp:
                    for pair in range(2):  # 0: cos q0/q1, 1: sin q2/q3
                        lc = locp.tile([128, 2, BPC, C], F32, tag="lc")
                        for cc in range(2):
                            off = (256 if pair == 0 else 0) + cc * 128
                            nc.tensor.matmul(
                                out=lc[:, cc], lhsT=wcon_sb[:, off:off + 128],
                                rhs=pos_r, start=True, stop=True,
                            )
                        # a = t + MAGIC rounds t to nearest int (RNE);
                        # (a - MAGIC) - t = -r, r in [-0.5, 0.5].
                        # sin(2pi*t) = -Sin(2pi*(-r)); the -1 lives in hti.
                        # The add runs on ACT (activation Copy bias) so the
                        # serial DVE chain is just the two fused subtracts.
                        aq = wsb.tile([128, 2, BPC, C], F32, tag="aq")
                        nc.scalar.activation(
                            out=aq, in_=lc,
                            func=mybir.ActivationFunctionType.Identity,
                            bias=magic_sb[:, :], scale=1.0,
                        )
                        negr = wsb.tile([128, 2, BPC, C], F32, tag="negr")
                        nc.vector.scalar_tensor_tensor(
                            out=negr, in0=aq, scalar=MAGIC, in1=lc,
                            op0=mybir.AluOpType.subtract,
                            op1=mybir.AluOpType.subtract,
                        )
                        nc.scalar.activation(
                            out=embq[:, 2 * pair:2 * pair + 2], in_=negr,
                            func=mybir.ActivationFunctionType.Sin,
                            scale=TWO_PI, bias=0.0,
                        )
                if debug:
                    nc.sync.dma_start(
                        out=emb_out[:, :],
                        in_=embq[:, :, 0, :],
                    )

                scores_ps = wps.tile([O, BPC, 128], F32, tag="scores")
                for b in range(BPC):
                    for q in range(4):
                        nc.tensor.matmul(
                            out=scores_ps[:, b, :],
                            lhsT=ht16_sb[:, q * O:(q + 1) * O],
                            rhs=embq[:, q, b, :],
                            start=(q == 0), stop=(q == 3),
                        )
                # scores are bounded (|s| < ~10): plain exp is fp32-safe and
                # softmax is shift-invariant, so skip the max-subtraction.
                # The exp table switch (~2.7us) overlaps the scores matmuls.
                probs = wsb.tile([O, BPC, 128], F32, tag="probs")
                ssum = wsb.tile([O, BPC], F32, tag="ssum")
                for b in range(BPC):
                    nc.scalar.activation(
                        out=probs[:, b, :], in_=scores_ps[:, b, :],
                        func=mybir.ActivationFunctionType.Exp,
                        bias=0.0, accum_out=ssum[:, b:b + 1],
                    )

                rcp = wsb.tile([O, BPC], F32, tag="rcp")
                nc.vector.reciprocal(out=rcp, in_=ssum)
                wgt = wsb.tile([O, BPC, 128], F16, tag="wgt")
                wt_ps = wps.tile([128, BPC, O], F16, tag="wtps")
                for b in range(BPC):
                    nc.vector.tensor_scalar_mul(
                        out=wgt[:, b, :], in0=probs[:, b, :],
                        scalar1=rcp[:, b:b + 1],
                    )
                    nc.tensor.transpose(
                        out=wt_ps[:, b, :], in_=wgt[:, b, :],
                        identity=ident16_sb[0:O, 0:O],
                    )
                nc.vector.tensor_copy(out=wt16_all, in_=wt_ps)
                if debug:
                    nc.gpsimd.dma_start(out=wt_out[:, :], in_=wt16_all)

                # ---------- main loop: out[t,o] = sum_c eeg[t,c]*w[o,c] ----
                # Per 2-group store chunk: transpose eeg tiles on the PE,
                # matmul against the per-batch weights, cast to fp16 on the
                # ACT copy out of PSUM, store 0.5 MB on HWDGE.
                # tile_wait_until floors the scheduler-sim readiness of every
                # main-loop instruction so the weights path above owns the
                # head of each engine's static order (real execution is still
                # purely semaphore-driven).
                with (
                    tc.tile_pool(name="otp", bufs=3, space="PSUM") as otp,
                    tc.tile_wait_until(0.030),
                ):
                    for b in range(BPC):
                        wt_b = wt16_all[:, b * O:(b + 1) * O]
                        for ch in range(NGROUP // SCH):
                            o_sb = osb.tile([128, SCH * JI * O], F16, tag="osb",
                                            name=f"o_{b}_{ch}")
                            for g2 in range(SCH):
                                g = ch * SCH + g2
                                out_ps = otp.tile([128, JI * O], F32, tag="outps")
                                eg = e_group(b, g)
                                et_ps = etp.tile([128, JI * 128], F16,
                                                 tag="etps")
                                for j in range(JI):
                                    nc.tensor.transpose(
                                        out=et_ps[:, j * 128:(j + 1) * 128],
                                        in_=eg[:, j * 128:(j + 1) * 128],
                                        identity=ident16_sb,
                                    )
                                et_sb = ets.tile([128, JI * 128], F16,
                                                 tag="etsb")
                                nc.vector.tensor_copy(out=et_sb, in_=et_ps)
                                for j in range(JI):
                                    nc.tensor.matmul(
                                        out=out_ps[:, j * O:(j + 1) * O],
                                        lhsT=et_sb[:, j * 128:(j + 1) * 128],
                                        rhs=wt_b,
                                        start=True, stop=True,
                                    )
                                nc.scalar.copy(
                                    out=o_sb[:, g2 * JI * O:(g2 + 1) * JI * O],
                                    in_=out_ps,
                                )
                            out_r = out[b].rearrange(
                                "(g p j) o -> p g (j o)", p=128, j=JI
                            )
                            nc.sync.dma_start(
                                out=out_r[:, ch * SCH:(ch + 1) * SCH, :],
                                in_=o_sb.rearrange("p (g x) -> p g x", g=SCH),
                            )
    nc.finalize()
    return nc


_NC_CACHE = None


def _get_nc():
    global _NC_CACHE
    if _NC_CACHE is None:
        _NC_CACHE = _build_nc()
    return _NC_CACHE


def _make_in_maps(eeg, positions, heads):
    lps, hti = _host_constants(np.asarray(heads, dtype=np.float32))
    eeg = np.asarray(eeg, dtype=np.float32)
    positions = np.asarray(positions, dtype=np.float32)
    in_maps = []
    for core in range(N_CORES):
        sl = slice(core * BPC, (core + 1) * BPC)
        in_maps.append({
            "eeg": np.ascontiguousarray(eeg[sl]),
            "wcon": _wcon(lps, positions[sl]),
            "hti": hti,
        })
    return in_maps


def kernel(eeg, positions, heads, sub=None, **_unused):
    from concourse.bass_utils import run_bass_kernel_spmd

    nc = _get_nc()
    in_maps = _make_in_maps(eeg, positions, heads)
    res = run_bass_kernel_spmd(nc, in_maps, list(range(N_CORES)))
    out = np.concatenate([res.results[c]["out"] for c in range(N_CORES)], axis=0)
    return out.astype(np.float32)
